# revision 36
# baseline (speedup 1.0000x reference)
"""GhostAttention (B=2, T=2048, C=2048, H=16) on 8 Trainium2 NeuronCores.

Sharding: tensor-parallel over heads (Megatron-style). Core c owns heads
{2c, 2c+1}: it gets the 256 matching rows of Wq/Wk/Wv (column-parallel) and
the 256 matching columns of Wo (row-parallel), computes QKV projections,
masked-relu attention and its partial output projection for both batches,
and writes a full-shape partial y. The host sums the 8 partials.

Per-core dataflow (q/k/score matmuls fp32r: fp32 storage, bf16-rate on the
PE; the noise-tolerant AV path runs with bf16 w/v tiles):
  phase 1: ONE merged pass over all B*T=4096 tokens in 16 tiles of 256.
           Per-tile PSUM is 6 half-bank tiles (q/k per head, v per token
           chunk) so the pool double-buffers in 6 banks and the ACT/DVE
           drains of tile n hide entirely under tile n+1's matmuls: the PE
           never stalls, which also keeps it at the ramped 2.4 GHz pstate
           (any gap drops it to 1.2 GHz for ~3us). 256-wide moving keeps
           f32r at full rate. xin DMA issue rotates over the Sync/ACT/DVE
           hwdge queues so no single sequencer's 565-667ns per-DMA config
           cost paces the stream.
  phase 2: S^T blocks (tk=128, tq=512) = k-stationary @ q-moving, emitted
           one block ahead of the AV consumer (pps bufs=2); diagonal blocks
           are windowed to the unmasked column range (>=256 wide to keep
           full f32r rate) and get a causal 0/1 mask multiply on DVE; AV
           accumulates v-stationary @ w-moving (bf16) into (hd, tq) PSUM;
           a 128x128 ones matmul accumulates the normalizer pre-broadcast
           across all partitions. The tail (eps-add, reciprocal, po*rec)
           runs entirely on DVE -- never on ACT -- so the relu chain that
           feeds the PE is never delayed; tails are flushed between the
           next group's attention emission and the current outproj, hiding
           them under attention PE work. Both batches run as one
           uninterrupted 8-group pipeline (A0 A1 O0 A2 O1 ... A7 O6 O7).
  phase 3: out-projection, attn-stationary @ Wo-moving -> (tok, o) PSUM,
           drained ACT/DVE alternating to an fp16 staging tile (halves the
           partial-y HBM write); non-final stores issue from the GpSimd
           SWDGE queue, the last group's stores rotate over four queues so
           the final DMA lands as early as possible.
Weights stream as k-slice singles (slices 0-1, issued on the ACT/DVE/
GpSimd queues in parallel so the first matmul only waits ~2.5us) then
pairs interleaved into tile 0's kk loop, always >=4 slices ahead.
"""

import math
import sys

if "/opt/trn_rl_repo" not in sys.path:
    sys.path.insert(0, "/opt/trn_rl_repo")

import numpy as np
from contextlib import ExitStack

import concourse.bass as bass
import concourse.mybir as mybir
import concourse.tile as tile
from concourse.bass import ts, ds
from concourse.bass_utils import run_bass_kernel_spmd
from concourse.vector_clock import ScopedClock, VectorClock


def _split_drain_and_barrier(self, tick_clock, wait_clock):
    # This image's walrus caps sem waits per instruction; split the Tile-tail
    # drain waits across single-wait SP nops instead.
    gc = tick_clock.global_clock
    n = len(gc)
    for proc in range(n):
        t = gc[proc]
        if t <= 0:
            continue
        vc = VectorClock([0] * n)
        vc.require_at_least(proc, t)
        nop_inst = self.nc.sync.nop()
        wait_clock.add_sem_waits(nop_inst.ins, ScopedClock({None: vc}))
    self.nc.sync.drain()
    self.nc.all_engine_barrier()
    assert self.sems is not None
    popped = self.nc._tile_sem_poison_stack.pop()
    assert popped is self._sem_poison
    self.nc.clear_and_free_semaphores(list(self.sems.allocated().values()))
    self.nc.all_engine_barrier()


tile.TileContext._drain_and_barrier = _split_drain_and_barrier

_ws_counter = [0]


def split_excess_waits(nc, max_waits=1):
    """Hoist extra per-instruction sem waits onto preceding same-engine NoOps
    (same queue => they execute, and therefore wait, before the instruction)."""
    for fn in nc.m.functions:
        for blk in fn.blocks:
            insts = list(blk.instructions)
            out = []
            changed = False
            for inst in insts:
                si = inst.sync_info
                if si is not None and si.on_wait and len(si.on_wait) > max_waits:
                    waits = list(si.on_wait)
                    extra, keep = waits[:-max_waits], waits[-max_waits:]
                    for s in range(0, len(extra), max_waits):
                        chunk = extra[s : s + max_waits]
                        _ws_counter[0] += 1
                        nop = mybir.InstNoOp(
                            name=f"I-ws-{_ws_counter[0]}",
                            engine=inst.engine,
                            ins=[],
                            outs=[],
                            sync_info=mybir.SyncInfo(on_wait=chunk, on_update=[]),
                        )
                        out.append(nop)
                    inst.sync_info = mybir.SyncInfo(
                        on_wait=keep, on_update=list(si.on_update)
                    )
                    changed = True
                out.append(inst)
            if changed:
                try:
                    blk.instructions[:] = out
                except Exception:
                    blk.set_instructions(out)
    return nc


B, T, C = 2, 2048, 2048
H = 16
HD = C // H  # 128
N_CORES = 8
H_PER_CORE = H // N_CORES  # 2
CH = HD * H_PER_CORE  # 256 channels per core
SCALE = 1.0 / math.sqrt(HD)
ATTN_BIAS = 0.1  # relu(scores - (-0.1)) = relu(scores + 0.1)
EPS = 1e-6

F32 = mybir.dt.float32
F32R = mybir.dt.float32r
F16 = mybir.dt.float16
BF16 = mybir.dt.bfloat16
AF = mybir.ActivationFunctionType

_NC_CACHE = None


def _build(split_waits=True):
    nc = bass.Bass("TRN2", debug=False)
    xT = nc.dram_tensor("xT", [C, B * T], F16, kind="ExternalInput")
    wq = nc.dram_tensor("wq", [C, CH], F16, kind="ExternalInput")
    wk = nc.dram_tensor("wk", [C, CH], F16, kind="ExternalInput")
    wv = nc.dram_tensor("wv", [C, CH], F16, kind="ExternalInput")
    wo = nc.dram_tensor("wo", [CH, C], F16, kind="ExternalInput")
    masks = nc.dram_tensor("masks", [4, 128, 512], F32, kind="ExternalInput")
    y = nc.dram_tensor("y", [B * T, C], F16, kind="ExternalOutput")

    KT = C // 128  # 16 contraction tiles
    NTOK = (B * T) // 256  # 16 token tiles for the merged projection pass
    NT = T // 512  # 4 query tiles of 512 per batch

    with tile.TileContext(nc) as tc, ExitStack() as ctx:
        consts = ctx.enter_context(tc.tile_pool(name="consts", bufs=1))
        qkvp = ctx.enter_context(tc.tile_pool(name="qkv", bufs=1))

        wq_re = wq.ap().rearrange("(k p) o -> p k o", p=128)
        wk_re = wk.ap().rearrange("(k p) o -> p k o", p=128)
        wv_re = wv.ap().rearrange("(k p) o -> p k o", p=128)

        wo_sb = consts.tile([128, H_PER_CORE, C], F16, name="wo_sb", tag="wo")
        wo_re = wo.ap().rearrange("(h p) o -> p h o", p=128)
        mask_sb = consts.tile([128, 4, 512], F32, name="mask_sb", tag="masks")

        def load_wo_masks(step):
            # spread the 2MB wo + masks loads over several tile boundaries on
            # the sync queue so their transfers never starve the xin stream
            if step == 1:
                nc.sync.dma_start(wo_sb[:, 0, :], wo_re[:, 0, :])
            elif step == 2:
                nc.sync.dma_start(wo_sb[:, 1, :], wo_re[:, 1, :])
            elif step == 3:
                nc.sync.dma_start(mask_sb[:, 0, :], masks.ap()[0])
                nc.sync.dma_start(mask_sb[:, 1, :], masks.ap()[1])
            elif step == 4:
                nc.sync.dma_start(mask_sb[:, 2, :], masks.ap()[2])
                nc.sync.dma_start(mask_sb[:, 3, :], masks.ap()[3])

        # bf16 ones are exact and load into the PE ~3x faster than f32r
        ones_sq = consts.tile([128, 128], BF16, name="ones_sq", tag="ones_sq")
        nc.vector.memset(ones_sq[:], 1.0)
        bias_sb = consts.tile([128, 1], F32, name="bias_sb", tag="bias")
        nc.vector.memset(bias_sb[:], ATTN_BIAS)
        eps_sb = consts.tile([128, 1], F32, name="eps_sb", tag="eps")
        nc.vector.memset(eps_sb[:], EPS)

        q_sb = qkvp.tile([128, B, H_PER_CORE, T], F16, name="q_sb", tag="q")
        k_sb = qkvp.tile([128, B, H_PER_CORE, T], F16, name="k_sb", tag="k")
        v_sb = qkvp.tile([128, (B * T) // 128, CH], BF16, name="v_sb", tag="v")

        xT_re = xT.ap().rearrange("(k p) t -> p k t", p=128)  # (128, KT, B*T)

        # ---- phase 1: merged QKV projections for all 4096 tokens ----
        # 512-token tiles: the 512-wide q/k matmuls (213ns) fully hide the
        # ~112ns f32r LDWEIGHTS; q/k PSUM is single-buffered but its drains
        # complete before the tile's last v matmuls retire (q0 accumulation
        # stops 8 matmuls before the tile ends), and v PSUM double-buffers,
        # so the PE rolls into tile n+1 with no stall.
        with (
            tc.tile_pool(name="wtile", bufs=1) as wgt,
            tc.tile_pool(name="xin", bufs=6) as xinp,
            tc.tile_pool(name="ps_qk", bufs=1, space="PSUM") as ppqk,
            tc.tile_pool(name="ps_v1", bufs=2, space="PSUM") as ppv,
        ):
            # k-slices 0 and 1 as singles (on three different hwdge queues so
            # the first matmuls only wait for their own 128KB slice), the
            # rest as pairs interleaved into tile 0's kk loop.
            wq_s = [
                consts.tile([128, CH], F16, name=f"wq_s{i}", tag=f"wqs{i}")
                for i in range(2)
            ]
            wk_s = [
                consts.tile([128, CH], F16, name=f"wk_s{i}", tag=f"wks{i}")
                for i in range(2)
            ]
            wv_s = [
                consts.tile([128, CH], F16, name=f"wv_s{i}", tag=f"wvs{i}")
                for i in range(2)
            ]
            NPAIR = (KT - 2) // 2  # 7 pairs covering slices 2..15
            wq_p, wk_p, wv_p = [], [], []
            for i in range(NPAIR):
                wq_p.append(
                    wgt.tile([128, 2, CH], F16, name=f"wq_sb{i}", tag=f"wq{i}")
                )
                wk_p.append(
                    wgt.tile([128, 2, CH], F16, name=f"wk_sb{i}", tag=f"wk{i}")
                )
                wv_p.append(
                    wgt.tile([128, 2, CH], F16, name=f"wv_sb{i}", tag=f"wv{i}")
                )

            def wsl(singles, pairs, kk):
                if kk < 2:
                    return singles[kk][:]
                return pairs[(kk - 2) // 2][:, kk % 2, :]

            def load_w_single(i):
                nc.scalar.dma_start(wq_s[i][:], wq_re[:, i, :])
                nc.sync.dma_start(wk_s[i][:], wk_re[:, i, :])
                nc.gpsimd.dma_start(wv_s[i][:], wv_re[:, i, :])

            def load_w_pair(i):
                sl = ds(2 + 2 * i, 2)
                nc.scalar.dma_start(wq_p[i][:], wq_re[:, sl, :])
                nc.sync.dma_start(wk_p[i][:], wk_re[:, sl, :])
                nc.gpsimd.dma_start(wv_p[i][:], wv_re[:, sl, :])

            # prefetch the first few x tiles ahead of the weight singles so
            # the first matmul's two inputs arrive in parallel ~9us in
            xin_pre = {}
            for kk in range(3):
                xt = xinp.tile([128, 512], F16, name="xin", tag="xin")
                (nc.sync, nc.scalar, nc.sync)[kk].dma_start(
                    xt[:], xT_re[:, kk, ds(0, 512)]
                )
                xin_pre[kk] = xt
            load_w_single(0)
            load_w_single(1)
            load_w_pair(0)
            load_w_pair(1)

            NTILE = (B * T) // 512  # 8 merged projection tiles
            for n in range(NTILE):
                b, t0 = n // (NTILE // B), 512 * (n % (NTILE // B))
                ps_q = [
                    ppqk.tile([128, 512], F32, name=f"ps_q{h}", tag=f"psq{h}")
                    for h in (0, 1)
                ]
                ps_k = [
                    ppqk.tile([128, 512], F32, name=f"ps_k{h}", tag=f"psk{h}")
                    for h in (0, 1)
                ]
                # v chunks pack two accumulation regions per PSUM bank:
                # start=True on the bank's first matmul zeroes the whole 2KB
                # zero region, the bank's single stop rides on its last
                ps_v = [
                    ppv.tile([128, 2, 256], F32, name=f"ps_v{i}", tag=f"psv{i}")
                    for i in (0, 1)
                ]
                for kk in range(KT):
                    if n == 0 and kk % 2 == 0 and kk // 2 + 2 < NPAIR:
                        load_w_pair(kk // 2 + 2)
                    if n == 0 and kk in xin_pre:
                        xin = xin_pre.pop(kk)
                    else:
                        xin = xinp.tile([128, 512], F16, name="xin", tag="xin")
                        # alternate the per-DMA sequencer config cost over
                        # the SP and ACT hwdge queues
                        qeng = (nc.sync, nc.scalar)[kk % 2]
                        qeng.dma_start(xin[:], xT_re[:, kk, ds(512 * n, 512)])
                    st, sp = kk == 0, kk == KT - 1
                    for h in (0, 1):
                        nc.tensor.matmul(
                            ps_q[h][:],
                            wsl(wq_s, wq_p, kk)[:, ts(h, 128)],
                            xin[:],
                            start=st,
                            stop=sp,
                        )
                        nc.tensor.matmul(
                            ps_k[h][:],
                            wsl(wk_s, wk_p, kk)[:, ts(h, 128)],
                            xin[:],
                            start=st,
                            stop=sp,
                        )
                    for c in range(4):
                        nc.tensor.matmul(
                            ps_v[c // 2][:, c % 2, :],
                            xin[:, ts(c, 128)],
                            wsl(wv_s, wv_p, kk)[:],
                            start=st and c % 2 == 0,
                            stop=sp and c % 2 == 1,
                        )
                # drains split ACT/DVE, ordered to match the next tile's
                # matmul emission order; q/k drains complete before this
                # tile's v matmuls retire, v drains relax into tile n+1
                # via the double-buffered v banks
                tsl = ds(t0, 512)
                with nc.allow_low_precision(reason="f32r/bf16 staging copies"):
                    nc.scalar.mul(q_sb[:, b, 0, tsl], ps_q[0][:], SCALE)
                    nc.vector.tensor_scalar_mul(
                        q_sb[:, b, 1, tsl], ps_q[1][:], SCALE
                    )
                    nc.scalar.copy(k_sb[:, b, 0, tsl], ps_k[0][:])
                    nc.vector.tensor_copy(k_sb[:, b, 1, tsl], ps_k[1][:])
                    nc.scalar.copy(v_sb[:, 4 * n : 4 * n + 2, :], ps_v[0][:])
                    nc.vector.tensor_copy(
                        v_sb[:, 4 * n + 2 : 4 * n + 4, :], ps_v[1][:]
                    )
                load_wo_masks(n)

        # ---- phases 2+3: attention + output projection, both batches ----
        with (
            tc.tile_pool(name="wtile2", bufs=4) as wp,
            tc.tile_pool(name="attn", bufs=2) as attnp,
            tc.tile_pool(name="yst", bufs=2) as ystp,
            tc.tile_pool(name="small", bufs=2) as smallp,
            tc.tile_pool(name="ps_s", bufs=2, space="PSUM") as pps,
            tc.tile_pool(name="ps_o", bufs=2, space="PSUM") as ppo,
            tc.tile_pool(name="ps_d", bufs=2, space="PSUM") as ppd,
            tc.tile_pool(name="ps_y", bufs=2, space="PSUM") as ppy,
        ):
            attn_tiles = {}
            pending_tails = []

            def flush_tails():
                for fn in pending_tails:
                    fn()
                pending_tails.clear()

            def s_win(j, i):
                # diagonal block r=i-4j: columns < 128r are fully masked;
                # fp16 moving runs at full rate at any width, so the
                # window is exact
                r = i - 4 * j
                return 128 * r if r >= 0 else 0

            def emit_s_blk(b, j, hh, i):
                w0 = s_win(j, i)
                psb = pps.tile([128, 512], F32, name="psb", tag="ps")
                nc.tensor.matmul(
                    psb[:, w0:512],
                    k_sb[:, b, hh, ds(128 * i, 128)],
                    q_sb[:, b, hh, ds(512 * j + w0, 512 - w0)],
                    start=True,
                    stop=True,
                )
                return psb

            def emit_attention(b, j):
                nblk = 4 * j + 4

                for hh in (0, 1):
                    po = ppo.tile([128, 512], F32, name="po", tag="po")
                    # 128x128 ones stationary -> every partition of pd
                    # holds the denominator row: the reciprocal and the
                    # po multiply need no cross-partition broadcast
                    pd = ppd.tile([128, 512], F32, name="pd", tag="pd")

                    psb_cur = emit_s_blk(b, j, hh, 0)
                    for i in range(nblk):
                        psb, psb_cur = psb_cur, (
                            emit_s_blk(b, j, hh, i + 1) if i + 1 < nblk else None
                        )
                        w0 = s_win(j, i)
                        r = i - 4 * j
                        w_t = wp.tile([128, 512], BF16, name="w_t", tag="w")
                        if r >= 0:  # diagonal block: causal mask
                            # mask*relu(S+b) == relu(mask*(S+b)) for 0/1 mask
                            tmp = wp.tile(
                                [128, 512], F32, name="wtmp", tag="wtmp"
                            )
                            nc.vector.scalar_tensor_tensor(
                                tmp[:, w0:512],
                                psb[:, w0:512],
                                ATTN_BIAS,
                                mask_sb[:, r, w0:512],
                                op0=mybir.AluOpType.add,
                                op1=mybir.AluOpType.mult,
                            )
                            nc.scalar.activation(
                                w_t[:, w0:512],
                                tmp[:, w0:512],
                                AF.Relu,
                                bias=0.0,
                                scale=1.0,
                            )
                        else:
                            nc.scalar.activation(
                                w_t[:], psb[:], AF.Relu, bias=bias_sb[:], scale=1.0
                            )
                        nc.tensor.matmul(
                            po[:, w0:512],
                            v_sb[:, (T // 128) * b + i, ts(hh, 128)],
                            w_t[:, w0:512],
                            start=i == 0,
                            stop=i == nblk - 1,
                        )
                        nc.tensor.matmul(
                            pd[:, w0:512],
                            ones_sq[:],
                            w_t[:, w0:512],
                            start=i == 0,
                            stop=i == nblk - 1,
                        )
                        if i == 1:
                            # previous tail lands here, behind this head's
                            # first relu/mask, so its chain hides under
                            # attention PE work without delaying the w_t
                            # pipeline warm-up
                            flush_tails()

                    def tail(j=j, hh=hh, po=po, pd=pd):
                        # 1/(den+EPS) as exp(-ln(den+EPS)) on ACT: no PE
                        # broadcast, no slow DVE reciprocal; the po multiply
                        # rides on DVE
                        lnd = smallp.tile([128, 512], F32, name="lnd", tag="lnd")
                        nc.scalar.activation(
                            lnd[:], pd[:], AF.Ln, bias=eps_sb[:], scale=1.0
                        )
                        rec = smallp.tile([128, 512], F32R, name="rec", tag="rec")
                        with nc.allow_low_precision(
                            reason="f32r normalizer feeds out-proj matmul"
                        ):
                            nc.scalar.activation(
                                rec[:], lnd[:], AF.Exp, bias=0.0, scale=-1.0
                            )
                        at = attnp.tile(
                            [128, 512], F32R, name=f"at{hh}", tag=f"attn{hh}"
                        )
                        with nc.allow_low_precision(
                            reason="f32r attn staging feeds out-proj matmul"
                        ):
                            nc.vector.tensor_mul(at[:], po[:], rec[:])
                        attn_tiles[(j, hh)] = at

                    pending_tails.append(tail)

            def emit_outproj(b, j, final=False):
                a0 = attn_tiles.pop((j, 0))
                a1 = attn_tiles.pop((j, 1))
                for s in range(4):
                    yst = ystp.tile([128, C], F16, name="yst", tag="yst")
                    for ot in range(4):
                        py = ppy.tile([128, 512], F32, name="py", tag="py")
                        nc.tensor.matmul(
                            py[:],
                            a0[:, ts(s, 128)],
                            wo_sb[:, 0, ts(ot, 512)],
                            start=True,
                            stop=False,
                        )
                        nc.tensor.matmul(
                            py[:],
                            a1[:, ts(s, 128)],
                            wo_sb[:, 1, ts(ot, 512)],
                            start=False,
                            stop=True,
                        )
                        with nc.allow_low_precision(
                            reason="fp16 partial-y staging halves HBM traffic"
                        ):
                            # alternate ACT/DVE so either engine's serial
                            # drain chain stays under the PE fill rate
                            if ot % 2 == 0:
                                nc.scalar.copy(yst[:, ts(ot, 512)], py[:])
                            else:
                                nc.vector.tensor_copy(yst[:, ts(ot, 512)], py[:])
                        if final:
                            # last group of the kernel: store per-ot,
                            # rotating over four queues so the final DMA
                            # is issued (and lands) as early as possible
                            qeng = (nc.sync, nc.gpsimd, nc.scalar)[
                                (4 * s + ot) % 3
                            ]
                            qeng.dma_start(
                                y.ap()[
                                    ds(T * b + 512 * j + 128 * s, 128),
                                    ds(512 * ot, 512),
                                ],
                                yst[:, ts(ot, 512)],
                            )
                    if not final:
                        nc.gpsimd.dma_start(
                            y.ap()[ds(T * b + 512 * j + 128 * s, 128), :], yst[:]
                        )

            # one uninterrupted pipeline over the 8 (b, j) groups
            groups = [(b, j) for b in range(B) for j in range(NT)]
            emit_attention(*groups[0])
            for gi in range(1, len(groups)):
                emit_attention(*groups[gi])
                if gi == len(groups) - 1:
                    # flush the last head's tail before this outproj so
                    # its chain hides under ~8us of outproj PE work
                    flush_tails()
                emit_outproj(*groups[gi - 1])
            flush_tails()
            emit_outproj(*groups[-1], final=True)
    if split_waits:
        split_excess_waits(nc)
    return nc


def _host_masks():
    p = np.arange(128, dtype=np.int32)[:, None]
    f = np.arange(512, dtype=np.int32)[None, :]
    return np.stack(
        [(f >= 128 * r + p).astype(np.float32) for r in range(4)], axis=0
    )


def kernel(x, Wq, Wk, Wv, Wo, _trace=False, _trace_kwargs=None):
    global _NC_CACHE
    x = np.ascontiguousarray(np.asarray(x, dtype=np.float32))
    Wq = np.asarray(Wq, dtype=np.float32)
    Wk = np.asarray(Wk, dtype=np.float32)
    Wv = np.asarray(Wv, dtype=np.float32)
    Wo = np.asarray(Wo, dtype=np.float32)

    if _NC_CACHE is None:
        _NC_CACHE = _build()
    nc = _NC_CACHE

    # x and the QKV projection weights ship as fp16: 11-bit mantissa keeps
    # score noise ~30x below bf16 (max rel err 1.9e-3 in emulation) while
    # halving the phase-1 HBM burst that otherwise starves the first tile
    xT = np.ascontiguousarray(x.reshape(B * T, C).T).astype(np.float16)
    masks = _host_masks()
    in_maps = []
    for c in range(N_CORES):
        sl = slice(CH * c, CH * (c + 1))
        in_maps.append(
            {
                "xT": xT,
                "wq": np.ascontiguousarray(Wq[sl, :].T).astype(np.float16),
                "wk": np.ascontiguousarray(Wk[sl, :].T).astype(np.float16),
                "wv": np.ascontiguousarray(Wv[sl, :].T).astype(np.float16),
                "wo": np.ascontiguousarray(Wo[:, sl].T).astype(np.float16),
                "masks": masks,
            }
        )

    res = run_bass_kernel_spmd(
        nc,
        in_maps,
        core_ids=list(range(N_CORES)),
        trace=_trace,
        **(_trace_kwargs or {}),
    )
    acc = np.zeros((B * T, C), dtype=np.float64)
    for c in range(N_CORES):
        acc += res.results[c]["y"].astype(np.float64)
    out = acc.astype(np.float32).reshape(B, T, C)
    if _trace:
        return out, res
    return out


# revision 37
# speedup vs baseline: 1.2169x; 1.2169x over previous
"""GhostAttention (B=2, T=2048, C=2048, H=16) on 8 Trainium2 NeuronCores.

Sharding: tensor-parallel over heads (Megatron-style). Core c owns heads
{2c, 2c+1}: it gets the 256 matching rows of Wq/Wk/Wv (column-parallel) and
the 256 matching columns of Wo (row-parallel), computes QKV projections,
masked-relu attention and its partial output projection for both batches,
and writes a full-shape partial y. The host sums the 8 partials.

Per-core dataflow (q/k/score matmuls fp32r: fp32 storage, bf16-rate on the
PE; the noise-tolerant AV path runs with bf16 w/v tiles):
  phase 1: ONE merged pass over all B*T=4096 tokens in 16 tiles of 256.
           Per-tile PSUM is 6 half-bank tiles (q/k per head, v per token
           chunk) so the pool double-buffers in 6 banks and the ACT/DVE
           drains of tile n hide entirely under tile n+1's matmuls: the PE
           never stalls, which also keeps it at the ramped 2.4 GHz pstate
           (any gap drops it to 1.2 GHz for ~3us). 256-wide moving keeps
           f32r at full rate. xin DMA issue rotates over the Sync/ACT/DVE
           hwdge queues so no single sequencer's 565-667ns per-DMA config
           cost paces the stream.
  phase 2: S^T blocks (tk=128, tq=512) = k-stationary @ q-moving, emitted
           one block ahead of the AV consumer (pps bufs=2); diagonal blocks
           are windowed to the unmasked column range (>=256 wide to keep
           full f32r rate) and get a causal 0/1 mask multiply on DVE; AV
           accumulates v-stationary @ w-moving (bf16) into (hd, tq) PSUM;
           a 128x128 ones matmul accumulates the normalizer pre-broadcast
           across all partitions. The tail (eps-add, reciprocal, po*rec)
           runs entirely on DVE -- never on ACT -- so the relu chain that
           feeds the PE is never delayed; tails are flushed between the
           next group's attention emission and the current outproj, hiding
           them under attention PE work. Both batches run as one
           uninterrupted 8-group pipeline (A0 A1 O0 A2 O1 ... A7 O6 O7).
  phase 3: out-projection, attn-stationary @ Wo-moving -> (tok, o) PSUM,
           drained ACT/DVE alternating to an fp16 staging tile (halves the
           partial-y HBM write); non-final stores issue from the GpSimd
           SWDGE queue, the last group's stores rotate over four queues so
           the final DMA lands as early as possible.
Weights stream as k-slice singles (slices 0-1, issued on the ACT/DVE/
GpSimd queues in parallel so the first matmul only waits ~2.5us) then
pairs interleaved into tile 0's kk loop, always >=4 slices ahead.
"""

import math
import sys

if "/opt/trn_rl_repo" not in sys.path:
    sys.path.insert(0, "/opt/trn_rl_repo")

import numpy as np
from contextlib import ExitStack

import concourse.bass as bass
import concourse.mybir as mybir
import concourse.tile as tile
from concourse.bass import ts, ds
from concourse.bass_utils import run_bass_kernel_spmd
from concourse.vector_clock import ScopedClock, VectorClock


def _split_drain_and_barrier(self, tick_clock, wait_clock):
    # This image's walrus caps sem waits per instruction; split the Tile-tail
    # drain waits across single-wait SP nops instead.
    gc = tick_clock.global_clock
    n = len(gc)
    for proc in range(n):
        t = gc[proc]
        if t <= 0:
            continue
        vc = VectorClock([0] * n)
        vc.require_at_least(proc, t)
        nop_inst = self.nc.sync.nop()
        wait_clock.add_sem_waits(nop_inst.ins, ScopedClock({None: vc}))
    self.nc.sync.drain()
    self.nc.all_engine_barrier()
    assert self.sems is not None
    popped = self.nc._tile_sem_poison_stack.pop()
    assert popped is self._sem_poison
    self.nc.clear_and_free_semaphores(list(self.sems.allocated().values()))
    self.nc.all_engine_barrier()


tile.TileContext._drain_and_barrier = _split_drain_and_barrier

_ws_counter = [0]


def split_excess_waits(nc, max_waits=1):
    """Hoist extra per-instruction sem waits onto preceding same-engine NoOps
    (same queue => they execute, and therefore wait, before the instruction)."""
    for fn in nc.m.functions:
        for blk in fn.blocks:
            insts = list(blk.instructions)
            out = []
            changed = False
            for inst in insts:
                si = inst.sync_info
                if si is not None and si.on_wait and len(si.on_wait) > max_waits:
                    waits = list(si.on_wait)
                    extra, keep = waits[:-max_waits], waits[-max_waits:]
                    for s in range(0, len(extra), max_waits):
                        chunk = extra[s : s + max_waits]
                        _ws_counter[0] += 1
                        nop = mybir.InstNoOp(
                            name=f"I-ws-{_ws_counter[0]}",
                            engine=inst.engine,
                            ins=[],
                            outs=[],
                            sync_info=mybir.SyncInfo(on_wait=chunk, on_update=[]),
                        )
                        out.append(nop)
                    inst.sync_info = mybir.SyncInfo(
                        on_wait=keep, on_update=list(si.on_update)
                    )
                    changed = True
                out.append(inst)
            if changed:
                try:
                    blk.instructions[:] = out
                except Exception:
                    blk.set_instructions(out)
    return nc


B, T, C = 2, 2048, 2048
H = 16
HD = C // H  # 128
N_CORES = 8
H_PER_CORE = H // N_CORES  # 2
CH = HD * H_PER_CORE  # 256 channels per core
SCALE = 1.0 / math.sqrt(HD)
ATTN_BIAS = 0.1  # relu(scores - (-0.1)) = relu(scores + 0.1)
EPS = 1e-6

F32 = mybir.dt.float32
F32R = mybir.dt.float32r
F16 = mybir.dt.float16
BF16 = mybir.dt.bfloat16
AF = mybir.ActivationFunctionType

_NC_CACHE = None


def _build(split_waits=True):
    nc = bass.Bass("TRN2", debug=False)
    xT = nc.dram_tensor("xT", [C, B * T], F16, kind="ExternalInput")
    wq = nc.dram_tensor("wq", [C, CH], F16, kind="ExternalInput")
    wk = nc.dram_tensor("wk", [C, CH], F16, kind="ExternalInput")
    wv = nc.dram_tensor("wv", [C, CH], F16, kind="ExternalInput")
    wo = nc.dram_tensor("wo", [CH, C], F16, kind="ExternalInput")
    masks = nc.dram_tensor("masks", [4, 128, 512], F32, kind="ExternalInput")
    y = nc.dram_tensor("y", [B * T, C], F16, kind="ExternalOutput")

    KT = C // 128  # 16 contraction tiles
    NTOK = (B * T) // 256  # 16 token tiles for the merged projection pass
    NT = T // 512  # 4 query tiles of 512 per batch

    with tile.TileContext(nc) as tc, ExitStack() as ctx:
        consts = ctx.enter_context(tc.tile_pool(name="consts", bufs=1))
        qkvp = ctx.enter_context(tc.tile_pool(name="qkv", bufs=1))

        wq_re = wq.ap().rearrange("(k p) o -> p k o", p=128)
        wk_re = wk.ap().rearrange("(k p) o -> p k o", p=128)
        wv_re = wv.ap().rearrange("(k p) o -> p k o", p=128)

        wo_sb = consts.tile([128, H_PER_CORE, C], F16, name="wo_sb", tag="wo")
        wo_re = wo.ap().rearrange("(h p) o -> p h o", p=128)
        mask_sb = consts.tile([128, 4, 512], F32, name="mask_sb", tag="masks")

        def load_wo_masks(step):
            # spread the 2MB wo + masks loads over several tile boundaries on
            # the sync queue so their transfers never starve the xin stream
            if step == 1:
                nc.sync.dma_start(wo_sb[:, 0, :], wo_re[:, 0, :])
            elif step == 2:
                nc.sync.dma_start(wo_sb[:, 1, :], wo_re[:, 1, :])
            elif step == 3:
                nc.sync.dma_start(mask_sb[:, 0, :], masks.ap()[0])
                nc.sync.dma_start(mask_sb[:, 1, :], masks.ap()[1])
            elif step == 4:
                nc.sync.dma_start(mask_sb[:, 2, :], masks.ap()[2])
                nc.sync.dma_start(mask_sb[:, 3, :], masks.ap()[3])

        # bf16 ones are exact and load into the PE ~3x faster than f32r
        ones_sq = consts.tile([128, 128], BF16, name="ones_sq", tag="ones_sq")
        nc.vector.memset(ones_sq[:], 1.0)
        bias_sb = consts.tile([128, 1], F32, name="bias_sb", tag="bias")
        nc.vector.memset(bias_sb[:], ATTN_BIAS)
        eps_sb = consts.tile([128, 1], F32, name="eps_sb", tag="eps")
        nc.vector.memset(eps_sb[:], EPS)

        q_sb = qkvp.tile([128, B, H_PER_CORE, T], F16, name="q_sb", tag="q")
        k_sb = qkvp.tile([128, B, H_PER_CORE, T], F16, name="k_sb", tag="k")
        v_sb = qkvp.tile([128, (B * T) // 128, CH], BF16, name="v_sb", tag="v")

        xT_re = xT.ap().rearrange("(k p) t -> p k t", p=128)  # (128, KT, B*T)

        # ---- phase 1: merged QKV projections for all 4096 tokens ----
        # 512-token tiles: the 512-wide q/k matmuls (213ns) fully hide the
        # ~112ns f32r LDWEIGHTS; q/k PSUM is single-buffered but its drains
        # complete before the tile's last v matmuls retire (q0 accumulation
        # stops 8 matmuls before the tile ends), and v PSUM double-buffers,
        # so the PE rolls into tile n+1 with no stall.
        with (
            tc.tile_pool(name="wtile", bufs=1) as wgt,
            tc.tile_pool(name="xin", bufs=6) as xinp,
            tc.tile_pool(name="ps_qk", bufs=1, space="PSUM") as ppqk,
            tc.tile_pool(name="ps_v1", bufs=2, space="PSUM") as ppv,
        ):
            # k-slices 0 and 1 as singles (on three different hwdge queues so
            # the first matmuls only wait for their own 128KB slice), the
            # rest as pairs interleaved into tile 0's kk loop.
            wq_s = [
                consts.tile([128, CH], F16, name=f"wq_s{i}", tag=f"wqs{i}")
                for i in range(2)
            ]
            wk_s = [
                consts.tile([128, CH], F16, name=f"wk_s{i}", tag=f"wks{i}")
                for i in range(2)
            ]
            wv_s = [
                consts.tile([128, CH], F16, name=f"wv_s{i}", tag=f"wvs{i}")
                for i in range(2)
            ]
            NPAIR = (KT - 2) // 2  # 7 pairs covering slices 2..15
            wq_p, wk_p, wv_p = [], [], []
            for i in range(NPAIR):
                wq_p.append(
                    wgt.tile([128, 2, CH], F16, name=f"wq_sb{i}", tag=f"wq{i}")
                )
                wk_p.append(
                    wgt.tile([128, 2, CH], F16, name=f"wk_sb{i}", tag=f"wk{i}")
                )
                wv_p.append(
                    wgt.tile([128, 2, CH], F16, name=f"wv_sb{i}", tag=f"wv{i}")
                )

            def wsl(singles, pairs, kk):
                if kk < 2:
                    return singles[kk][:]
                return pairs[(kk - 2) // 2][:, kk % 2, :]

            def load_w_single(i):
                nc.scalar.dma_start(wq_s[i][:], wq_re[:, i, :])
                nc.sync.dma_start(wk_s[i][:], wk_re[:, i, :])
                nc.gpsimd.dma_start(wv_s[i][:], wv_re[:, i, :])

            def load_w_pair(i):
                sl = ds(2 + 2 * i, 2)
                nc.scalar.dma_start(wq_p[i][:], wq_re[:, sl, :])
                nc.sync.dma_start(wk_p[i][:], wk_re[:, sl, :])
                nc.gpsimd.dma_start(wv_p[i][:], wv_re[:, sl, :])

            # prefetch the first few x tiles ahead of the weight singles so
            # the first matmul's two inputs arrive in parallel ~9us in
            xin_pre = {}
            for kk in range(3):
                xt = xinp.tile([128, 512], F16, name="xin", tag="xin")
                (nc.sync, nc.scalar, nc.sync)[kk].dma_start(
                    xt[:], xT_re[:, kk, ds(0, 512)]
                )
                xin_pre[kk] = xt
            load_w_single(0)
            load_w_single(1)
            load_w_pair(0)
            load_w_pair(1)

            NTILE = (B * T) // 512  # 8 merged projection tiles
            for n in range(NTILE):
                b, t0 = n // (NTILE // B), 512 * (n % (NTILE // B))
                ps_q = [
                    ppqk.tile([128, 512], F32, name=f"ps_q{h}", tag=f"psq{h}")
                    for h in (0, 1)
                ]
                ps_k = [
                    ppqk.tile([128, 512], F32, name=f"ps_k{h}", tag=f"psk{h}")
                    for h in (0, 1)
                ]
                # v chunks pack two accumulation regions per PSUM bank:
                # start=True on the bank's first matmul zeroes the whole 2KB
                # zero region, the bank's single stop rides on its last
                ps_v = [
                    ppv.tile([128, 2, 256], F32, name=f"ps_v{i}", tag=f"psv{i}")
                    for i in (0, 1)
                ]
                for kk in range(KT):
                    if n == 0 and kk % 2 == 0 and kk // 2 + 2 < NPAIR:
                        load_w_pair(kk // 2 + 2)
                    if n == 0 and kk in xin_pre:
                        xin = xin_pre.pop(kk)
                    else:
                        xin = xinp.tile([128, 512], F16, name="xin", tag="xin")
                        # alternate the per-DMA sequencer config cost over
                        # the SP and ACT hwdge queues
                        qeng = (nc.sync, nc.scalar)[kk % 2]
                        qeng.dma_start(xin[:], xT_re[:, kk, ds(512 * n, 512)])
                    st, sp = kk == 0, kk == KT - 1
                    for h in (0, 1):
                        nc.tensor.matmul(
                            ps_q[h][:],
                            wsl(wq_s, wq_p, kk)[:, ts(h, 128)],
                            xin[:],
                            start=st,
                            stop=sp,
                        )
                        nc.tensor.matmul(
                            ps_k[h][:],
                            wsl(wk_s, wk_p, kk)[:, ts(h, 128)],
                            xin[:],
                            start=st,
                            stop=sp,
                        )
                    for c in range(4):
                        nc.tensor.matmul(
                            ps_v[c // 2][:, c % 2, :],
                            xin[:, ts(c, 128)],
                            wsl(wv_s, wv_p, kk)[:],
                            start=st and c % 2 == 0,
                            stop=sp and c % 2 == 1,
                        )
                # drains split ACT/DVE, ordered to match the next tile's
                # matmul emission order; q/k drains complete before this
                # tile's v matmuls retire, v drains relax into tile n+1
                # via the double-buffered v banks
                tsl = ds(t0, 512)
                with nc.allow_low_precision(reason="f32r/bf16 staging copies"):
                    nc.scalar.mul(q_sb[:, b, 0, tsl], ps_q[0][:], SCALE)
                    nc.vector.tensor_scalar_mul(
                        q_sb[:, b, 1, tsl], ps_q[1][:], SCALE
                    )
                    nc.scalar.copy(k_sb[:, b, 0, tsl], ps_k[0][:])
                    nc.vector.tensor_copy(k_sb[:, b, 1, tsl], ps_k[1][:])
                    nc.scalar.copy(v_sb[:, 4 * n : 4 * n + 2, :], ps_v[0][:])
                    nc.vector.tensor_copy(
                        v_sb[:, 4 * n + 2 : 4 * n + 4, :], ps_v[1][:]
                    )
                load_wo_masks(n)

        # ---- phases 2+3: attention + output projection, both batches ----
        with (
            tc.tile_pool(name="wtile2", bufs=4) as wp,
            tc.tile_pool(name="attn", bufs=2) as attnp,
            tc.tile_pool(name="yst", bufs=2) as ystp,
            tc.tile_pool(name="small", bufs=2) as smallp,
            tc.tile_pool(name="ps_s", bufs=2, space="PSUM") as pps,
            tc.tile_pool(name="ps_o", bufs=2, space="PSUM") as ppo,
            tc.tile_pool(name="ps_d", bufs=2, space="PSUM") as ppd,
            tc.tile_pool(name="ps_y", bufs=2, space="PSUM") as ppy,
        ):
            attn_tiles = {}
            pending_tails = []

            def flush_tails():
                for fn in pending_tails:
                    fn()
                pending_tails.clear()

            def s_win(j, i):
                # diagonal block r=i-4j: columns < 128r are fully masked;
                # fp16 moving runs at full rate at any width, so the
                # window is exact
                r = i - 4 * j
                return 128 * r if r >= 0 else 0

            def emit_s_blk(b, j, hh, i):
                w0 = s_win(j, i)
                psb = pps.tile([128, 512], F32, name="psb", tag="ps")
                nc.tensor.matmul(
                    psb[:, w0:512],
                    k_sb[:, b, hh, ds(128 * i, 128)],
                    q_sb[:, b, hh, ds(512 * j + w0, 512 - w0)],
                    start=True,
                    stop=True,
                )
                return psb

            def emit_attention(b, j):
                nblk = 4 * j + 4

                for hh in (0, 1):
                    po = ppo.tile([128, 512], F32, name="po", tag="po")
                    # 128x128 ones stationary -> every partition of pd
                    # holds the denominator row: the reciprocal and the
                    # po multiply need no cross-partition broadcast
                    pd = ppd.tile([128, 512], F32, name="pd", tag="pd")

                    psb_cur = emit_s_blk(b, j, hh, 0)
                    for i in range(nblk):
                        psb, psb_cur = psb_cur, (
                            emit_s_blk(b, j, hh, i + 1) if i + 1 < nblk else None
                        )
                        w0 = s_win(j, i)
                        r = i - 4 * j
                        w_t = wp.tile([128, 512], BF16, name="w_t", tag="w")
                        if r >= 0:  # diagonal block: causal mask
                            # mask*relu(S+b) == relu(mask*(S+b)) for 0/1 mask
                            tmp = wp.tile(
                                [128, 512], F32, name="wtmp", tag="wtmp"
                            )
                            nc.vector.scalar_tensor_tensor(
                                tmp[:, w0:512],
                                psb[:, w0:512],
                                ATTN_BIAS,
                                mask_sb[:, r, w0:512],
                                op0=mybir.AluOpType.add,
                                op1=mybir.AluOpType.mult,
                            )
                            nc.scalar.activation(
                                w_t[:, w0:512],
                                tmp[:, w0:512],
                                AF.Relu,
                                bias=0.0,
                                scale=1.0,
                            )
                        else:
                            nc.scalar.activation(
                                w_t[:], psb[:], AF.Relu, bias=bias_sb[:], scale=1.0
                            )
                        nc.tensor.matmul(
                            po[:, w0:512],
                            v_sb[:, (T // 128) * b + i, ts(hh, 128)],
                            w_t[:, w0:512],
                            start=i == 0,
                            stop=i == nblk - 1,
                        )
                        nc.tensor.matmul(
                            pd[:, w0:512],
                            ones_sq[:],
                            w_t[:, w0:512],
                            start=i == 0,
                            stop=i == nblk - 1,
                        )
                        if i == 1:
                            # previous tail lands here, behind this head's
                            # first relu/mask, so its chain hides under
                            # attention PE work without delaying the w_t
                            # pipeline warm-up
                            flush_tails()

                    def tail(j=j, hh=hh, po=po, pd=pd):
                        # 1/(den+EPS) as exp(-ln(den+EPS)) on ACT: no PE
                        # broadcast, no slow DVE reciprocal; the po multiply
                        # rides on DVE
                        lnd = smallp.tile([128, 512], F32, name="lnd", tag="lnd")
                        nc.scalar.activation(
                            lnd[:], pd[:], AF.Ln, bias=eps_sb[:], scale=1.0
                        )
                        rec = smallp.tile([128, 512], F32R, name="rec", tag="rec")
                        with nc.allow_low_precision(
                            reason="f32r normalizer feeds out-proj matmul"
                        ):
                            nc.scalar.activation(
                                rec[:], lnd[:], AF.Exp, bias=0.0, scale=-1.0
                            )
                        at = attnp.tile(
                            [128, 512], F16, name=f"at{hh}", tag=f"attn{hh}"
                        )
                        with nc.allow_low_precision(
                            reason="f32r attn staging feeds out-proj matmul"
                        ):
                            nc.vector.tensor_mul(at[:], po[:], rec[:])
                        attn_tiles[(j, hh)] = at

                    pending_tails.append(tail)

            def emit_outproj(b, j, final=False):
                a0 = attn_tiles.pop((j, 0))
                a1 = attn_tiles.pop((j, 1))
                for s in range(4):
                    yst = ystp.tile([128, C], F16, name="yst", tag="yst")
                    for ot in range(4):
                        py = ppy.tile([128, 512], F32, name="py", tag="py")
                        nc.tensor.matmul(
                            py[:],
                            a0[:, ts(s, 128)],
                            wo_sb[:, 0, ts(ot, 512)],
                            start=True,
                            stop=False,
                        )
                        nc.tensor.matmul(
                            py[:],
                            a1[:, ts(s, 128)],
                            wo_sb[:, 1, ts(ot, 512)],
                            start=False,
                            stop=True,
                        )
                        with nc.allow_low_precision(
                            reason="fp16 partial-y staging halves HBM traffic"
                        ):
                            # alternate ACT/DVE so either engine's serial
                            # drain chain stays under the PE fill rate
                            if ot % 2 == 0:
                                nc.scalar.copy(yst[:, ts(ot, 512)], py[:])
                            else:
                                nc.vector.tensor_copy(yst[:, ts(ot, 512)], py[:])
                        if final:
                            # last group of the kernel: store per-ot,
                            # rotating over four queues so the final DMA
                            # is issued (and lands) as early as possible
                            qeng = (nc.sync, nc.gpsimd, nc.scalar)[
                                (4 * s + ot) % 3
                            ]
                            qeng.dma_start(
                                y.ap()[
                                    ds(T * b + 512 * j + 128 * s, 128),
                                    ds(512 * ot, 512),
                                ],
                                yst[:, ts(ot, 512)],
                            )
                    if not final:
                        nc.gpsimd.dma_start(
                            y.ap()[ds(T * b + 512 * j + 128 * s, 128), :], yst[:]
                        )

            # one uninterrupted pipeline over the 8 (b, j) groups
            groups = [(b, j) for b in range(B) for j in range(NT)]
            emit_attention(*groups[0])
            for gi in range(1, len(groups)):
                emit_attention(*groups[gi])
                if gi == len(groups) - 1:
                    # flush the last head's tail before this outproj so
                    # its chain hides under ~8us of outproj PE work
                    flush_tails()
                emit_outproj(*groups[gi - 1])
            flush_tails()
            emit_outproj(*groups[-1], final=True)
    if split_waits:
        split_excess_waits(nc)
    return nc


def _host_masks():
    p = np.arange(128, dtype=np.int32)[:, None]
    f = np.arange(512, dtype=np.int32)[None, :]
    return np.stack(
        [(f >= 128 * r + p).astype(np.float32) for r in range(4)], axis=0
    )


def kernel(x, Wq, Wk, Wv, Wo, _trace=False, _trace_kwargs=None):
    global _NC_CACHE
    x = np.ascontiguousarray(np.asarray(x, dtype=np.float32))
    Wq = np.asarray(Wq, dtype=np.float32)
    Wk = np.asarray(Wk, dtype=np.float32)
    Wv = np.asarray(Wv, dtype=np.float32)
    Wo = np.asarray(Wo, dtype=np.float32)

    if _NC_CACHE is None:
        _NC_CACHE = _build()
    nc = _NC_CACHE

    # x and the QKV projection weights ship as fp16: 11-bit mantissa keeps
    # score noise ~30x below bf16 (max rel err 1.9e-3 in emulation) while
    # halving the phase-1 HBM burst that otherwise starves the first tile
    xT = np.ascontiguousarray(x.reshape(B * T, C).T).astype(np.float16)
    masks = _host_masks()
    in_maps = []
    for c in range(N_CORES):
        sl = slice(CH * c, CH * (c + 1))
        in_maps.append(
            {
                "xT": xT,
                "wq": np.ascontiguousarray(Wq[sl, :].T).astype(np.float16),
                "wk": np.ascontiguousarray(Wk[sl, :].T).astype(np.float16),
                "wv": np.ascontiguousarray(Wv[sl, :].T).astype(np.float16),
                "wo": np.ascontiguousarray(Wo[:, sl].T).astype(np.float16),
                "masks": masks,
            }
        )

    res = run_bass_kernel_spmd(
        nc,
        in_maps,
        core_ids=list(range(N_CORES)),
        trace=_trace,
        **(_trace_kwargs or {}),
    )
    acc = np.zeros((B * T, C), dtype=np.float64)
    for c in range(N_CORES):
        acc += res.results[c]["y"].astype(np.float64)
    out = acc.astype(np.float32).reshape(B, T, C)
    if _trace:
        return out, res
    return out
